# revision 12
# baseline (speedup 1.0000x reference)
"""Trainium2 Bass kernel for a GQA attention layer (B=2, S=2048, D=4096,
32 q-heads, 8 kv-heads, HD=128, RoPE, causal mask).

Sharding: 8 cores = 2 (batch) x 4 (head groups). Each core handles one
batch and 8 q-heads / 2 kv-heads: column-parallel wq/wk/wv, row-parallel
wo. Each core emits a partial [S, D] output (bf16); the host sums the 4
partials per batch in f32. No collectives.

Device dataflow (per core):
  phase 1: QKV projections from host-pretransposed xT (feature-major),
           RoPE applied in a "split" head layout per head right after its
           psum evacuation.
  phase 2: scoresT[sk,sq] tiles are computed in PAIRS into a 2-bank psum
           tile so one ScalarE exp covers [128,1024]; probs are fp16.
           The softmax denominator is a running fp16 sum on the DVE
           (acc += probs tile, 2x mode) finished by a single ones-matmul
           per (head, sq-block) that also broadcasts it to 128 psum
           partitions; 1/denom applied during attn psum evacuation.
           PV accumulates with V fp16 (token-major) stationary, trailing
           the exp stream by a 2-pair skew.
  phase 3: out_partial = attnT^T x wo-rows, streamed per 512-col block.
           Projection panels and oproj units are interleaved INTO the
           attention head stream (panel n+1 fills attn block n; oproj
           fills block 3) so the PE never waits on the exp stream.
"""

import sys

if "/opt/trn_rl_repo" not in sys.path:
    sys.path.insert(0, "/opt/trn_rl_repo")

import math
from contextlib import ExitStack

import ml_dtypes
import numpy as np

import concourse.bass as bass  # noqa: F401  (AP types used implicitly)
import concourse.tile as tile
from concourse import bacc, mybir
from concourse.bass_utils import run_bass_kernel_spmd

BF16 = ml_dtypes.bfloat16
F32 = mybir.dt.float32
BF = mybir.dt.bfloat16
F16 = mybir.dt.float16

B, S, D = 2, 2048, 4096
NH, NKV, HD = 32, 8, 128
G = 4  # head groups -> cores per batch
HPG = NH // G  # 8 q heads per core
KPG = NKV // G  # 2 kv heads per core
SCALE = 1.0 / math.sqrt(HD)

NFT = D // 128  # 32 feature tiles (contraction)
PTOK = 512  # token panel width in phase 1
NPANEL = S // PTOK  # 4
NTT = S // 128  # 16 token tiles
NSQ = S // 512  # 4 sq tiles
NOD = D // 512  # 8 out-D tiles

_CACHE = {}


def _build_program(phases=(1, 2, 3), reps=1):
    nc = bacc.Bacc("TRN2", target_bir_lowering=False, debug=False, num_devices=8)

    xt = nc.dram_tensor("xt", [D, S], BF, kind="ExternalInput").ap()
    wq = nc.dram_tensor("wq", [HPG, 128, NFT * 128], BF, kind="ExternalInput").ap()
    wk = nc.dram_tensor("wk", [KPG, 128, NFT * 128], BF, kind="ExternalInput").ap()
    wv = nc.dram_tensor("wv", [128, NFT * KPG * 128], BF, kind="ExternalInput").ap()
    wo = nc.dram_tensor("wo", [NOD, 128, HPG * 512], BF, kind="ExternalInput").ap()
    cosb = nc.dram_tensor("cosb", [128, S], BF, kind="ExternalInput").ap()
    sinb = nc.dram_tensor("sinb", [128, S], BF, kind="ExternalInput").ap()
    diagm = nc.dram_tensor("diagm", [128, 128], F32, kind="ExternalInput").ap()
    ones = nc.dram_tensor("ones", [128, 128], F16, kind="ExternalInput").ap()
    outp = nc.dram_tensor("outp", [S, D], BF, kind="ExternalOutput").ap()

    EXP = mybir.ActivationFunctionType.Exp
    MULT = mybir.AluOpType.mult

    with tile.TileContext(nc) as tc, ExitStack() as ctx:
        pool = lambda name, bufs: ctx.enter_context(tc.tile_pool(name=name, bufs=bufs))
        ppool = lambda name, bufs: ctx.enter_context(
            tc.tile_pool(name=name, bufs=bufs, space="PSUM")
        )

        persist = pool("persist", 1)
        xpool = pool("xpool", 4)
        wqpool = pool("wqpool", 3)
        ropepool = pool("ropepool", 2)
        probpool = pool("probpool", 4)
        accpool = pool("accpool", 3)
        bigden = pool("bigden", 2)
        wopool = pool("wopool", 3)
        outpool = pool("outpool", 4)

        psS = ppool("psS", 2)  # [128,1024] f32 (2 banks): paired score tiles
        psAt = ppool("psAt", 2)  # [128,512] f32: attn accum per head
        psB = ppool("psB", 2)  # [128,512] f32: proj/oproj/denominator bcast

        # ---- persistent tiles ----
        qt = [persist.tile([128, S], BF, tag=f"qt{h}", name=f"qt{h}") for h in range(HPG)]
        kt = [persist.tile([128, S], BF, tag=f"kt{k}", name=f"kt{k}") for k in range(KPG)]
        v_sb = persist.tile([128, NTT * KPG * 128], F16, tag="v", name="v_sb")
        v_w_sb = persist.tile([128, NFT * KPG * 128], BF, tag="vw", name="v_w_sb")
        at = [persist.tile([128, S], BF, tag=f"at{h}", name=f"at{h}") for h in range(HPG)]
        cos_sb = persist.tile([128, S], BF, tag="cos", name="cos_sb")
        sin_sb = persist.tile([128, S], BF, tag="sin", name="sin_sb")
        diag_sb = persist.tile([128, 128], F32, tag="diag", name="diag_sb")
        ones_sb = persist.tile([128, 128], F16, tag="ones", name="ones_sb")

        do1, do2, do3 = (1 in phases), (2 in phases), (3 in phases)
        xt_v = xt.rearrange("(f p) t -> p f t", p=128)
        qk_dst = list(qt) + list(kt)

        def emit_consts_a():
            nc.sync.dma_start(cos_sb[:], cosb[:])
            nc.sync.dma_start(sin_sb[:], sinb[:])

        def emit_consts_b():
            nc.sync.dma_start(v_w_sb[:], wv[:])
            nc.sync.dma_start(diag_sb[:], diagm[:])
            nc.sync.dma_start(ones_sb[:], ones[:])

        def panel_units(n, consts=False):
            """Generator of emission units for projection panel n."""
            tok0 = n * PTOK
            pre_wh = None
            if n == 0:
                # wh0 first so the PE's first LDWEIGHTS isn't behind 4 MB of x
                pre_wh = wqpool.tile([128, NFT * 128], BF, tag="wqt", name="wh")
                nc.sync.dma_start(pre_wh[:], wq[0])
            halves = []
            for q4 in range(4):
                xq = xpool.tile([128, 8 * PTOK], BF, tag="xts", name="xq")
                nc.sync.dma_start(
                    xq.rearrange("p (f t) -> p f t", t=PTOK),
                    xt_v[:, q4 * 8 : (q4 + 1) * 8, tok0 : tok0 + PTOK],
                )
                halves.append((xq, q4 * 8))
            yield

            def load_wh(hh):
                wsrc = wq[hh] if hh < HPG else wk[hh - HPG]
                wh = wqpool.tile([128, NFT * 128], BF, tag="wqt", name="wh")
                nc.sync.dma_start(wh[:], wsrc)
                return wh

            wh_next = pre_wh if pre_wh is not None else load_wh(0)
            for hh in range(HPG + KPG):
                wh = wh_next
                wh_next = load_wh(hh + 1) if hh + 1 < HPG + KPG else None
                if consts and hh == 0:
                    emit_consts_a()
                elif consts and hh == 3:
                    emit_consts_b()
                ps = psB.tile([128, PTOK], F32, tag="psB", name="ps_qk")
                for xtile, f0 in halves:
                    for fl in range(8):
                        f = f0 + fl
                        nc.tensor.matmul(
                            ps[:],
                            wh[:, f * 128 : (f + 1) * 128],
                            xtile[:, fl * PTOK : (fl + 1) * PTOK],
                            start=(f == 0),
                            stop=(f == NFT - 1),
                        )
                dst = qk_dst[hh]
                nc.vector.tensor_copy(dst[:, tok0 : tok0 + PTOK], ps[:])
                # RoPE this slice: dst = dst*C + swap_halves(dst)*S2
                rsw = ropepool.tile([128, PTOK], BF, tag="rsw", name="rsw")
                nc.sync.dma_start(rsw[0:64, :], dst[64:128, tok0 : tok0 + PTOK])
                nc.sync.dma_start(rsw[64:128, :], dst[0:64, tok0 : tok0 + PTOK])
                nc.vector.tensor_mul(rsw[:], rsw[:], sin_sb[:, tok0 : tok0 + PTOK])
                nc.vector.tensor_mul(
                    dst[:, tok0 : tok0 + PTOK],
                    dst[:, tok0 : tok0 + PTOK],
                    cos_sb[:, tok0 : tok0 + PTOK],
                )
                nc.vector.tensor_add(
                    dst[:, tok0 : tok0 + PTOK],
                    dst[:, tok0 : tok0 + PTOK],
                    rsw[:],
                )
                yield

            for m in range(PTOK // 128):
                ps = psB.tile([128, PTOK], F32, tag="psB", name="ps_v")
                for xtile, f0 in halves:
                    for fl in range(8):
                        f = f0 + fl
                        nc.tensor.matmul(
                            ps[:, 0 : KPG * 128],
                            xtile[:, fl * PTOK + m * 128 : fl * PTOK + m * 128 + 128],
                            v_w_sb[:, f * 256 : (f + 1) * 256],
                            start=(f == 0),
                            stop=(f == NFT - 1),
                        )
                tglob = n * (PTOK // 128) + m
                nc.vector.tensor_copy(
                    v_sb[:, tglob * 256 : (tglob + 1) * 256], ps[:, 0 : KPG * 128]
                )
                yield

        def oproj_unit(d, m, wod):
            ps = psB.tile([128, 512], F32, tag="psB", name="ps_o")
            for h in range(HPG):
                nc.tensor.matmul(
                    ps[:],
                    at[h][:, m * 128 : (m + 1) * 128],
                    wod[:, h * 512 : (h + 1) * 512],
                    start=(h == 0),
                    stop=(h == HPG - 1),
                )
            osb = outpool.tile([128, 512], BF, tag="osb", name="osb")
            nc.vector.tensor_copy(osb[:], ps[:])
            nc.sync.dma_start(
                outp[m * 128 : (m + 1) * 128, d * 512 : (d + 1) * 512], osb[:]
            )

        def attn_block(j, filler):
            """Attention for sq block j; pulls PE filler units between heads."""
            sq0 = j * 512
            n_sk = 4 * (j + 1)
            npairs = n_sk // 2
            PSKEW = 2
            fin_prev = None

            for h in range(HPG):
                kv = h // (HPG // KPG)
                acc = accpool.tile([128, 512], F16, tag="acc", name="acc")
                ps_a = psAt.tile([128, 512], F32, tag="psAt", name="psAt_t")
                pend = {}
                for pi in range(npairs + PSKEW):
                    if pi < npairs:
                        ps = psS.tile([128, 1024], F32, tag="psS", name="psS_t")
                        offs = []
                        for k in range(2):
                            t = 2 * pi + k
                            r = t - 4 * j
                            off = 128 * r if r >= 0 else 0
                            offs.append(off)
                            base = 512 * k
                            nc.tensor.matmul(
                                ps[:, base + off : base + 512],
                                kt[kv][:, t * 128 : (t + 1) * 128],
                                qt[h][:, sq0 + off : sq0 + 512],
                                start=True,
                                stop=True,
                            )
                            if r >= 0:
                                nc.vector.tensor_add(
                                    ps[:, base + off : base + off + 128],
                                    ps[:, base + off : base + off + 128],
                                    diag_sb[:],
                                )
                        pt = probpool.tile([128, 1024], F16, tag="probs", name="probs_t")
                        if offs[0] == 0 and offs[1] == 0:
                            nc.scalar.activation(pt[:], ps[:], EXP, scale=SCALE)
                        else:
                            for k in range(2):
                                o = 512 * k + offs[k]
                                nc.scalar.activation(
                                    pt[:, o : 512 * k + 512],
                                    ps[:, o : 512 * k + 512],
                                    EXP,
                                    scale=SCALE,
                                )
                        for k in range(2):
                            t = 2 * pi + k
                            off = offs[k]
                            base = 512 * k
                            if t == 0:
                                nc.vector.tensor_copy(acc[:], pt[:, 0:512])
                            else:
                                nc.vector.tensor_add(
                                    acc[:, off:512],
                                    acc[:, off:512],
                                    pt[:, base + off : base + 512],
                                )
                        pend[pi] = (pt, offs)
                        if pi == 0 and fin_prev is not None:
                            fin_prev()
                            fin_prev = None
                    if pi >= PSKEW:
                        pt, offs = pend.pop(pi - PSKEW)
                        for k in range(2):
                            t = 2 * (pi - PSKEW) + k
                            off = offs[k]
                            base = 512 * k
                            nc.tensor.matmul(
                                ps_a[:, off:512],
                                v_sb[:, t * 256 + kv * 128 : t * 256 + kv * 128 + 128],
                                pt[:, base + off : base + 512],
                                start=(t == 0),
                                stop=(t == n_sk - 1),
                            )

                def make_fin(h=h, acc=acc, ps_a=ps_a):
                    def fin():
                        ps_d = psB.tile([128, 512], F32, tag="psB", name="ps_d")
                        nc.tensor.matmul(
                            ps_d[:], ones_sb[:], acc[:], start=True, stop=True
                        )
                        inv_b = bigden.tile([128, 512], F32, tag="inv_b", name="inv_b")
                        nc.vector.reciprocal(inv_b[:], ps_d[:])
                        nc.vector.tensor_tensor(
                            at[h][:, sq0 : sq0 + 512], ps_a[:], inv_b[:], MULT
                        )
                    return fin

                fin_prev = make_fin()
                # pull a couple of filler units to keep PE fed while ACT works
                for _ in range(2):
                    if filler is not None:
                        try:
                            next(filler)
                        except StopIteration:
                            filler = None
            fin_prev()
            # drain remaining filler units
            if filler is not None:
                for _ in filler:
                    pass

        def oproj_block3_filler():
            """oproj units for d=0, m=0..7 (at[] ready from blocks 0/1)."""
            wod = wopool.tile([128, HPG * 512], BF, tag="wot", name="wod")
            nc.sync.dma_start(wod[:], wo[0])
            for m in range(HPG):
                oproj_unit(0, m, wod)
                yield
            _PENDING_WOD[0] = wod

        _PENDING_WOD = {}

        def oproj_rest():
            wod0 = _PENDING_WOD.pop(0, None)
            if wod0 is not None:
                nxt = wopool.tile([128, HPG * 512], BF, tag="wot", name="wod")
                nc.sync.dma_start(nxt[:], wo[1])
                for m in range(HPG, NTT):
                    oproj_unit(0, m, wod0)
                cur, d_start = nxt, 1
            else:
                cur = wopool.tile([128, HPG * 512], BF, tag="wot", name="wod")
                nc.sync.dma_start(cur[:], wo[0])
                d_start = 0
            for d in range(d_start, NOD):
                # prefetch next d-block's weights before this block's matmuls
                if d + 1 < NOD:
                    nxt = wopool.tile([128, HPG * 512], BF, tag="wot", name="wod")
                    nc.sync.dma_start(nxt[:], wo[d + 1])
                else:
                    nxt = None
                for m in range(NTT):
                    oproj_unit(d, m, cur)
                cur = nxt

        for _rep in range(reps):
            if do1:
                p0 = panel_units(0, consts=(_rep == 0))
                for _ in p0:
                    pass
            for j in range(NPANEL):
                if not do2:
                    if do1 and j < NPANEL - 1:
                        for _ in panel_units(j + 1):
                            pass
                    continue
                if j < NPANEL - 1:
                    filler = panel_units(j + 1) if do1 else None
                elif do3:
                    filler = oproj_block3_filler()
                else:
                    filler = None
                attn_block(j, filler)
            if do3:
                oproj_rest()

    nc.compile()
    return nc


_SPLIT_PERM = np.concatenate([np.arange(0, HD, 2), np.arange(1, HD, 2)])


def _host_prep(x, freqs_cos, freqs_sin, mask, wq, wk, wv, wo):
    """Build per-core input maps (8 cores = 2 batches x 4 head groups)."""
    x = np.asarray(x, np.float32)
    wq = np.asarray(wq, np.float32)
    wk = np.asarray(wk, np.float32)
    wv = np.asarray(wv, np.float32)
    wo = np.asarray(wo, np.float32)
    freqs_cos = np.asarray(freqs_cos, np.float32)
    freqs_sin = np.asarray(freqs_sin, np.float32)
    mask = np.asarray(mask, np.float32)

    xts = [np.ascontiguousarray(x[b].T).astype(BF16) for b in range(B)]

    ct = freqs_cos.T  # [64, S]
    st = freqs_sin.T
    cosb = np.concatenate([ct, ct], axis=0).astype(BF16)
    sinb = np.concatenate([-st, st], axis=0).astype(BF16)
    diagm = np.ascontiguousarray(
        mask[0:128, 0:128].T * math.sqrt(HD), dtype=np.float32
    )
    ones = np.ones((128, 128), np.float16)

    per_g = []
    for g in range(G):
        wq_g = wq[:, g * HPG * HD : (g + 1) * HPG * HD].reshape(D, HPG, HD)
        wq_g = wq_g[:, :, _SPLIT_PERM]
        wq_g = np.ascontiguousarray(
            wq_g.reshape(NFT, 128, HPG, HD).transpose(2, 1, 0, 3).reshape(HPG, 128, NFT * 128)
        ).astype(BF16)

        wk_g = wk[:, g * KPG * HD : (g + 1) * KPG * HD].reshape(D, KPG, HD)
        wk_g = wk_g[:, :, _SPLIT_PERM]
        wk_g = np.ascontiguousarray(
            wk_g.reshape(NFT, 128, KPG, HD).transpose(2, 1, 0, 3).reshape(KPG, 128, NFT * 128)
        ).astype(BF16)

        wv_g = np.ascontiguousarray(
            wv[:, g * KPG * HD : (g + 1) * KPG * HD]
            .reshape(NFT, 128, KPG * 128)
            .transpose(1, 0, 2)
            .reshape(128, NFT * KPG * 128)
        ).astype(BF16)

        wo_g = wo[g * HPG * HD : (g + 1) * HPG * HD, :]
        wo_g = np.ascontiguousarray(
            wo_g.reshape(HPG, 128, NOD, 512).transpose(2, 1, 0, 3).reshape(NOD, 128, HPG * 512)
        ).astype(BF16)

        per_g.append((wq_g, wk_g, wv_g, wo_g))

    in_maps = []
    for core in range(8):
        b, g = divmod(core, G)
        wq_g, wk_g, wv_g, wo_g = per_g[g]
        in_maps.append(
            {
                "xt": xts[b],
                "wq": wq_g,
                "wk": wk_g,
                "wv": wv_g,
                "wo": wo_g,
                "cosb": cosb,
                "sinb": sinb,
                "diagm": diagm,
                "ones": ones,
            }
        )
    return in_maps


def get_program(phases=(1, 2, 3), reps=1):
    key = ("nc", tuple(phases), reps)
    if key not in _CACHE:
        _CACHE[key] = _build_program(phases, reps)
    return _CACHE[key]


def kernel(
    x, start_pos, freqs_cos, freqs_sin, mask, wq, wk, wv, wo, **_ignored
):
    nc = get_program()
    in_maps = _host_prep(x, freqs_cos, freqs_sin, mask, wq, wk, wv, wo)
    res = run_bass_kernel_spmd(nc, in_maps, core_ids=list(range(8)))
    partials = [res.results[c]["outp"].astype(np.float32) for c in range(8)]
    out = np.stack(
        [
            partials[b * G]
            + partials[b * G + 1]
            + partials[b * G + 2]
            + partials[b * G + 3]
            for b in range(B)
        ]
    ).astype(np.float32)
    return out


# revision 29
# speedup vs baseline: 1.9544x; 1.9544x over previous
"""Trainium2 Bass kernel for a GQA attention layer (B=2, S=2048, D=4096,
32 q-heads, 8 kv-heads, HD=128, RoPE, causal mask).

Sharding: 8 cores = 2 (batch) x 4 (head groups). Each core handles one
batch and 8 q-heads / 2 kv-heads: column-parallel wq/wk/wv, row-parallel
wo. Each core emits a partial [S, D] output (bf16); the host sums the 4
partials per batch in f32. No collectives.

Device dataflow (per core):
  phase 1: QKV projections from host-pretransposed xT (feature-major),
           RoPE applied in a "split" head layout per head right after its
           psum evacuation.
  phase 2: scoresT[sk,sq] tiles are computed in PAIRS into a 2-bank psum
           tile so one ScalarE exp covers [128,1024]; probs are fp16.
           The softmax denominator is a running fp16 sum on the DVE
           (acc += probs tile, 2x mode) finished by a single ones-matmul
           per (head, sq-block) that also broadcasts it to 128 psum
           partitions; 1/denom applied during attn psum evacuation.
           PV accumulates with V fp16 (token-major) stationary, trailing
           the exp stream by a 2-pair skew.
  phase 3: out_partial = attnT^T x wo-rows, streamed per 512-col block.
           Projection panels and oproj units are interleaved INTO the
           attention head stream (panel n+1 fills attn block n; oproj
           fills block 3) so the PE never waits on the exp stream.
"""

import sys

if "/opt/trn_rl_repo" not in sys.path:
    sys.path.insert(0, "/opt/trn_rl_repo")

import math
from contextlib import ExitStack

import ml_dtypes
import numpy as np

import concourse.bass as bass  # noqa: F401  (AP types used implicitly)
import concourse.tile as tile
from concourse import bacc, mybir
from concourse.bass_utils import run_bass_kernel_spmd

BF16 = ml_dtypes.bfloat16
F32 = mybir.dt.float32
BF = mybir.dt.bfloat16
F16 = mybir.dt.float16

B, S, D = 2, 2048, 4096
NH, NKV, HD = 32, 8, 128
G = 4  # head groups -> cores per batch
HPG = NH // G  # 8 q heads per core
KPG = NKV // G  # 2 kv heads per core
SCALE = 1.0 / math.sqrt(HD)

NFT = D // 128  # 32 feature tiles (contraction)
PTOK = 512  # token panel width in phase 1
NPANEL = S // PTOK  # 4
NTT = S // 128  # 16 token tiles
NSQ = S // 512  # 4 sq tiles
NOD = D // 512  # 8 out-D tiles

_CACHE = {}


def _build_program(phases=(1, 2, 3), reps=1):
    nc = bacc.Bacc("TRN2", target_bir_lowering=False, debug=False, num_devices=8)

    xt = nc.dram_tensor("xt", [D, S], BF, kind="ExternalInput").ap()
    wq = nc.dram_tensor("wq", [HPG, 128, NFT * 128], BF, kind="ExternalInput").ap()
    wk = nc.dram_tensor("wk", [KPG, 128, NFT * 128], BF, kind="ExternalInput").ap()
    wv = nc.dram_tensor("wv", [128, NFT * KPG * 128], BF, kind="ExternalInput").ap()
    wo = nc.dram_tensor("wo", [NOD, 128, HPG * 512], BF, kind="ExternalInput").ap()
    cosb = nc.dram_tensor("cosb", [128, S], BF, kind="ExternalInput").ap()
    sinb = nc.dram_tensor("sinb", [128, S], BF, kind="ExternalInput").ap()
    diagm = nc.dram_tensor("diagm", [128, 128], BF, kind="ExternalInput").ap()
    ident = nc.dram_tensor("ident", [128, 128], BF, kind="ExternalInput").ap()
    ones = nc.dram_tensor("ones", [128, 128], F16, kind="ExternalInput").ap()
    outp = nc.dram_tensor("outp", [S, D], BF, kind="ExternalOutput").ap()

    EXP = mybir.ActivationFunctionType.Exp
    MULT = mybir.AluOpType.mult

    with tile.TileContext(nc) as tc, ExitStack() as ctx:
        pool = lambda name, bufs: ctx.enter_context(tc.tile_pool(name=name, bufs=bufs))
        ppool = lambda name, bufs: ctx.enter_context(
            tc.tile_pool(name=name, bufs=bufs, space="PSUM")
        )

        persist = pool("persist", 1)
        xpool = pool("xpool", 4)
        wqpool = pool("wqpool", 3)
        ropepool = pool("ropepool", 2)
        probpool = pool("probpool", 4)
        accpool = pool("accpool", 3)
        bigden = pool("bigden", 2)
        wopool = pool("wopool", 3)
        outpool = pool("outpool", 4)

        psS = ppool("psS", 2)  # [128,1024] f32 (2 banks): paired score tiles
        psAt = ppool("psAt", 2)  # [128,512] f32: attn accum per head
        psB = ppool("psB", 2)  # [128,512] f32: proj/oproj/denominator bcast

        # ---- persistent tiles ----
        qt = [persist.tile([128, S], BF, tag=f"qt{h}", name=f"qt{h}") for h in range(HPG)]
        kt = [persist.tile([128, S], BF, tag=f"kt{k}", name=f"kt{k}") for k in range(KPG)]
        v_sb = persist.tile([128, NTT * KPG * 128], F16, tag="v", name="v_sb")
        v_w_sb = persist.tile([128, NFT * KPG * 128], BF, tag="vw", name="v_w_sb")
        at = [persist.tile([128, S], BF, tag=f"at{h}", name=f"at{h}") for h in range(HPG)]
        cos_sb = persist.tile([128, S], BF, tag="cos", name="cos_sb")
        sin_sb = persist.tile([128, S], BF, tag="sin", name="sin_sb")
        diag_sb = persist.tile([128, 128], BF, tag="diag", name="diag_sb")
        ident_sb = persist.tile([128, 128], BF, tag="ident", name="ident_sb")
        ones_sb = persist.tile([128, 128], F16, tag="ones", name="ones_sb")

        do1, do2, do3 = (1 in phases), (2 in phases), (3 in phases)
        xt_v = xt.rearrange("(f p) t -> p f t", p=128)
        # K heads first so attention block j never waits on the K rope
        qk_dst = list(kt) + list(qt)

        def emit_consts_a():
            nc.sync.dma_start(cos_sb[:], cosb[:])
            nc.sync.dma_start(sin_sb[:], sinb[:])

        def emit_consts_b():
            nc.sync.dma_start(v_w_sb[:], wv[:])
            nc.sync.dma_start(diag_sb[:], diagm[:])
            nc.sync.dma_start(ident_sb[:], ident[:])
            nc.sync.dma_start(ones_sb[:], ones[:])

        def panel_units(n, consts=False):
            """Generator of emission units for projection panel n."""
            tok0 = n * PTOK
            pre_wh = None
            if n == 0:
                # wh0 first so the PE's first LDWEIGHTS isn't behind 4 MB of x
                pre_wh = wqpool.tile([128, NFT * 128], BF, tag="wqt", name="wh")
                nc.sync.dma_start(pre_wh[:], wk[0])
            halves = []
            for q4 in range(4):
                xq = xpool.tile([128, 8 * PTOK], BF, tag="xts", name="xq")
                nc.sync.dma_start(
                    xq.rearrange("p (f t) -> p f t", t=PTOK),
                    xt_v[:, q4 * 8 : (q4 + 1) * 8, tok0 : tok0 + PTOK],
                )
                halves.append((xq, q4 * 8))
            yield

            def load_wh(hh):
                wsrc = wk[hh] if hh < KPG else wq[hh - KPG]
                wh = wqpool.tile([128, NFT * 128], BF, tag="wqt", name="wh")
                nc.sync.dma_start(wh[:], wsrc)
                return wh

            wh_next = pre_wh if pre_wh is not None else load_wh(0)
            for hh in range(HPG + KPG):
                wh = wh_next
                wh_next = load_wh(hh + 1) if hh + 1 < HPG + KPG else None
                if consts and hh == 0:
                    emit_consts_a()
                elif consts and hh == 3:
                    emit_consts_b()
                ps = psB.tile([128, PTOK], F32, tag="psB", name="ps_qk")
                for xtile, f0 in halves:
                    for fl in range(8):
                        f = f0 + fl
                        nc.tensor.matmul(
                            ps[:],
                            wh[:, f * 128 : (f + 1) * 128],
                            xtile[:, fl * PTOK : (fl + 1) * PTOK],
                            start=(f == 0),
                            stop=(f == NFT - 1),
                        )
                dst = qk_dst[hh]
                nc.vector.tensor_copy(dst[:, tok0 : tok0 + PTOK], ps[:])
                # RoPE this slice: dst = dst*C + swap_halves(dst)*S2
                rsw = ropepool.tile([128, PTOK], BF, tag="rsw", name="rsw")
                nc.sync.dma_start(rsw[0:64, :], dst[64:128, tok0 : tok0 + PTOK])
                nc.sync.dma_start(rsw[64:128, :], dst[0:64, tok0 : tok0 + PTOK])
                nc.vector.tensor_mul(rsw[:], rsw[:], sin_sb[:, tok0 : tok0 + PTOK])
                nc.vector.tensor_mul(
                    dst[:, tok0 : tok0 + PTOK],
                    dst[:, tok0 : tok0 + PTOK],
                    cos_sb[:, tok0 : tok0 + PTOK],
                )
                nc.vector.tensor_add(
                    dst[:, tok0 : tok0 + PTOK],
                    dst[:, tok0 : tok0 + PTOK],
                    rsw[:],
                )
                yield

            for m in range(PTOK // 128):
                ps = psB.tile([128, PTOK], F32, tag="psB", name="ps_v")
                for xtile, f0 in halves:
                    for fl in range(8):
                        f = f0 + fl
                        nc.tensor.matmul(
                            ps[:, 0 : KPG * 128],
                            xtile[:, fl * PTOK + m * 128 : fl * PTOK + m * 128 + 128],
                            v_w_sb[:, f * 256 : (f + 1) * 256],
                            start=(f == 0),
                            stop=(f == NFT - 1),
                        )
                tglob = n * (PTOK // 128) + m
                nc.vector.tensor_copy(
                    v_sb[:, tglob * 256 : (tglob + 1) * 256], ps[:, 0 : KPG * 128]
                )
                yield

        def oproj_unit(d, m, wod):
            ps = psB.tile([128, 512], F32, tag="psB", name="ps_o")
            for h in range(HPG):
                nc.tensor.matmul(
                    ps[:],
                    at[h][:, m * 128 : (m + 1) * 128],
                    wod[:, h * 512 : (h + 1) * 512],
                    start=(h == 0),
                    stop=(h == HPG - 1),
                )
            osb = outpool.tile([128, 512], BF, tag="osb", name="osb")
            nc.vector.tensor_copy(osb[:], ps[:])
            nc.sync.dma_start(
                outp[m * 128 : (m + 1) * 128, d * 512 : (d + 1) * 512], osb[:]
            )

        def attn_block(j, filler, n_filler=0):
            """Attention for sq block j; pulls PE filler units between heads."""
            sq0 = j * 512
            n_sk = 4 * (j + 1)
            npairs = n_sk // 2
            PSKEW = 2
            fin_prev = None
            pulled = 0

            for h in range(HPG):
                kv = h // (HPG // KPG)
                acc = accpool.tile([128, 512], F16, tag="acc", name="acc")
                ps_a = psAt.tile([128, 512], F32, tag="psAt", name="psAt_t")
                pend = {}
                accq = []

                def emit_acc(pa, pt_a, offs_a):
                    for k in range(2):
                        t = 2 * pa + k
                        off = offs_a[k]
                        base = 512 * k
                        if t == 0:
                            nc.vector.tensor_copy(acc[:], pt_a[:, 0:512])
                        else:
                            nc.vector.tensor_add(
                                acc[:, off:512],
                                acc[:, off:512],
                                pt_a[:, base + off : base + 512],
                            )
                for pi in range(npairs + PSKEW):
                    if pi < npairs:
                        ps = psS.tile([128, 1024], F32, tag="psS", name="psS_t")
                        offs = []
                        for k in range(2):
                            t = 2 * pi + k
                            r = t - 4 * j
                            off = 128 * r if r >= 0 else 0
                            offs.append(off)
                            base = 512 * k
                            nc.tensor.matmul(
                                ps[:, base + off : base + 512],
                                kt[kv][:, t * 128 : (t + 1) * 128],
                                qt[h][:, sq0 + off : sq0 + 512],
                                start=True,
                                stop=(r < 0),
                            )
                            if r >= 0:
                                # causal mask via accumulating matmul:
                                # ps[:, diag 128 cols] += I^T @ maskT
                                nc.tensor.matmul(
                                    ps[:, base + off : base + off + 128],
                                    ident_sb[:],
                                    diag_sb[:],
                                    start=False,
                                    stop=True,
                                )
                        pt = probpool.tile([128, 1024], F16, tag="probs", name="probs_t")
                        if offs[0] == 0 and offs[1] == 0:
                            nc.scalar.activation(pt[:], ps[:], EXP, scale=SCALE)
                        else:
                            for k in range(2):
                                o = 512 * k + offs[k]
                                nc.scalar.activation(
                                    pt[:, o : 512 * k + 512],
                                    ps[:, o : 512 * k + 512],
                                    EXP,
                                    scale=SCALE,
                                )
                        pend[pi] = (pt, offs)
                        accq.append((pi, pt, offs))
                        if pi == 0 and fin_prev is not None:
                            fin_prev()
                            fin_prev = None
                    # acc adds run one pair late so the next pair's diag adds
                    # aren't stuck behind them in the in-order DVE queue
                    # (diag -> exp -> PV is the critical path)
                    while accq and (len(accq) > 1 or pi >= npairs - 1):
                        emit_acc(*accq.pop(0))
                    if pi >= PSKEW:
                        pt, offs = pend.pop(pi - PSKEW)
                        for k in range(2):
                            t = 2 * (pi - PSKEW) + k
                            off = offs[k]
                            base = 512 * k
                            nc.tensor.matmul(
                                ps_a[:, off:512],
                                v_sb[:, t * 256 + kv * 128 : t * 256 + kv * 128 + 128],
                                pt[:, base + off : base + 512],
                                start=(t == 0),
                                stop=(t == n_sk - 1),
                            )

                def make_fin(h=h, acc=acc, ps_a=ps_a):
                    def fin():
                        ps_d = psB.tile([128, 512], F32, tag="psB", name="ps_d")
                        nc.tensor.matmul(
                            ps_d[:], ones_sb[:], acc[:], start=True, stop=True
                        )
                        inv_b = bigden.tile([128, 512], F32, tag="inv_b", name="inv_b")
                        nc.vector.reciprocal(inv_b[:], ps_d[:])
                        nc.vector.tensor_tensor(
                            at[h][:, sq0 : sq0 + 512], ps_a[:], inv_b[:], MULT
                        )
                    return fin

                fin_prev = make_fin()
                # spread filler units evenly across heads to keep PE fed
                want = (n_filler * (h + 1) + HPG - 1) // HPG
                while pulled < want and filler is not None:
                    try:
                        next(filler)
                        pulled += 1
                    except StopIteration:
                        filler = None
            fin_prev()
            # drain remaining filler units
            if filler is not None:
                for _ in filler:
                    pass

        def oproj_block3_filler():
            """oproj units for d=0, m=0..7 (at[] ready from blocks 0/1)."""
            wod = wopool.tile([128, HPG * 512], BF, tag="wot", name="wod")
            nc.sync.dma_start(wod[:], wo[0])
            for m in range(HPG):
                oproj_unit(0, m, wod)
                yield
            _PENDING_WOD[0] = wod

        _PENDING_WOD = {}

        def oproj_rest():
            wod0 = _PENDING_WOD.pop(0, None)
            if wod0 is not None:
                nxt = wopool.tile([128, HPG * 512], BF, tag="wot", name="wod")
                nc.sync.dma_start(nxt[:], wo[1])
                for m in range(HPG, NTT):
                    oproj_unit(0, m, wod0)
                cur, d_start = nxt, 1
            else:
                cur = wopool.tile([128, HPG * 512], BF, tag="wot", name="wod")
                nc.sync.dma_start(cur[:], wo[0])
                d_start = 0
            for d in range(d_start, NOD):
                # prefetch next d-block's weights before this block's matmuls
                if d + 1 < NOD:
                    nxt = wopool.tile([128, HPG * 512], BF, tag="wot", name="wod")
                    nc.sync.dma_start(nxt[:], wo[d + 1])
                else:
                    nxt = None
                for m in range(NTT):
                    oproj_unit(d, m, cur)
                cur = nxt

        for _rep in range(reps):
            if do1:
                p0 = panel_units(0, consts=(_rep == 0))
                for _ in p0:
                    pass
            for j in range(NPANEL):
                if not do2:
                    if do1 and j < NPANEL - 1:
                        for _ in panel_units(j + 1):
                            pass
                    continue
                if j < NPANEL - 1:
                    filler = panel_units(j + 1) if do1 else None
                    n_filler = 15
                elif do3:
                    filler = oproj_block3_filler()
                    n_filler = 8
                else:
                    filler = None
                    n_filler = 0
                attn_block(j, filler, n_filler)
            if do3:
                oproj_rest()

    nc.compile()
    return nc


_SPLIT_PERM = np.concatenate([np.arange(0, HD, 2), np.arange(1, HD, 2)])


def _host_prep(x, freqs_cos, freqs_sin, mask, wq, wk, wv, wo):
    """Build per-core input maps (8 cores = 2 batches x 4 head groups)."""
    x = np.asarray(x, np.float32)
    wq = np.asarray(wq, np.float32)
    wk = np.asarray(wk, np.float32)
    wv = np.asarray(wv, np.float32)
    wo = np.asarray(wo, np.float32)
    freqs_cos = np.asarray(freqs_cos, np.float32)
    freqs_sin = np.asarray(freqs_sin, np.float32)
    mask = np.asarray(mask, np.float32)

    xts = [np.ascontiguousarray(x[b].T).astype(BF16) for b in range(B)]

    ct = freqs_cos.T  # [64, S]
    st = freqs_sin.T
    cosb = np.concatenate([ct, ct], axis=0).astype(BF16)
    sinb = np.concatenate([-st, st], axis=0).astype(BF16)
    diagm = np.ascontiguousarray(mask[0:128, 0:128].T * math.sqrt(HD)).astype(BF16)
    ident = np.eye(128, dtype=np.float32).astype(BF16)
    ones = np.ones((128, 128), np.float16)

    per_g = []
    for g in range(G):
        wq_g = wq[:, g * HPG * HD : (g + 1) * HPG * HD].reshape(D, HPG, HD)
        wq_g = wq_g[:, :, _SPLIT_PERM]
        wq_g = np.ascontiguousarray(
            wq_g.reshape(NFT, 128, HPG, HD).transpose(2, 1, 0, 3).reshape(HPG, 128, NFT * 128)
        ).astype(BF16)

        wk_g = wk[:, g * KPG * HD : (g + 1) * KPG * HD].reshape(D, KPG, HD)
        wk_g = wk_g[:, :, _SPLIT_PERM]
        wk_g = np.ascontiguousarray(
            wk_g.reshape(NFT, 128, KPG, HD).transpose(2, 1, 0, 3).reshape(KPG, 128, NFT * 128)
        ).astype(BF16)

        wv_g = np.ascontiguousarray(
            wv[:, g * KPG * HD : (g + 1) * KPG * HD]
            .reshape(NFT, 128, KPG * 128)
            .transpose(1, 0, 2)
            .reshape(128, NFT * KPG * 128)
        ).astype(BF16)

        wo_g = wo[g * HPG * HD : (g + 1) * HPG * HD, :]
        wo_g = np.ascontiguousarray(
            wo_g.reshape(HPG, 128, NOD, 512).transpose(2, 1, 0, 3).reshape(NOD, 128, HPG * 512)
        ).astype(BF16)

        per_g.append((wq_g, wk_g, wv_g, wo_g))

    in_maps = []
    for core in range(8):
        b, g = divmod(core, G)
        wq_g, wk_g, wv_g, wo_g = per_g[g]
        in_maps.append(
            {
                "xt": xts[b],
                "wq": wq_g,
                "wk": wk_g,
                "wv": wv_g,
                "wo": wo_g,
                "cosb": cosb,
                "sinb": sinb,
                "diagm": diagm,
                "ident": ident,
                "ones": ones,
            }
        )
    return in_maps


def get_program(phases=(1, 2, 3), reps=1):
    key = ("nc", tuple(phases), reps)
    if key not in _CACHE:
        _CACHE[key] = _build_program(phases, reps)
    return _CACHE[key]


def kernel(
    x, start_pos, freqs_cos, freqs_sin, mask, wq, wk, wv, wo, **_ignored
):
    nc = get_program()
    in_maps = _host_prep(x, freqs_cos, freqs_sin, mask, wq, wk, wv, wo)
    res = run_bass_kernel_spmd(nc, in_maps, core_ids=list(range(8)))
    partials = [res.results[c]["outp"].astype(np.float32) for c in range(8)]
    out = np.stack(
        [
            partials[b * G]
            + partials[b * G + 1]
            + partials[b * G + 2]
            + partials[b * G + 3]
            for b in range(B)
        ]
    ).astype(np.float32)
    return out
